# revision 20
# baseline (speedup 1.0000x reference)
"""Trainium2 Bass kernel for nn_DMN (Dynamic Memory Network).

Data-parallel over batch B=32 across 8 NeuronCores (4 rows/core).
Per core: fully-unrolled GRU scans (context S=1024, question SQ=32) in a
gates-on-partition layout ([6 gate-tiles of 128] x [4 batch]), input
projections precomputed as one big matmul, sentence-end gather via
DRAM round-trip + dma_gather(transpose=True), 3 attention hops, and the
answer module. Weights/activations in bf16, accumulation in fp32 PSUM.
"""

import sys

sys.path.insert(0, "/opt/trn_rl_repo")

import numpy as np

import concourse.bacc as bacc
import concourse.bass as bass
import concourse.tile as tile
from concourse.tile import add_dep_helper
from concourse import mybir
from concourse.bass_utils import run_bass_kernel_spmd

f32 = mybir.dt.float32
bf16 = mybir.dt.bfloat16
i16 = mybir.dt.int16
AF = mybir.ActivationFunctionType
ALU = mybir.AluOpType

# model dims (hardcoded per spec)
B, S_FULL, SQ_FULL, N, DIN, H, A, O = 32, 1024, 32, 128, 128, 256, 100, 2000
MEM_COUNT, ANS_COUNT = 3, 2
NCORES = 8
BL = B // NCORES           # 4 batch rows per core
G3 = 3 * H                 # 768 gates
NMT = G3 // 128            # 6 gate M-tiles
KH = H // 128              # 2 hidden K-tiles
OT = 16                    # output M-tiles (2000 -> 16 x 128, last ragged 80)
OT_LAST = O - 15 * 128     # 80
NEG = -1.0e38


def _ap(base, off, dims):
    """View into tile `base` (an AP) at element offset `off` with explicit
    free dims [[step, count], ...]; keeps base's partition dim."""
    return bass.AP(tensor=base.tensor, offset=base.offset + off,
                   ap=[list(base.ap[0])] + [list(d) for d in dims])


def _app(base, poff, pcount, off, dims):
    """Same but overriding partition range [poff, poff+pcount)."""
    pstep = base.ap[0][0]
    return bass.AP(tensor=base.tensor, offset=base.offset + poff * pstep + off,
                   ap=[[pstep, pcount]] + [list(d) for d in dims])


def _emit_gru_cell_gates(nc, work, psA, psB, gin_ap, hprev_ap, hout_ap, tag):
    """Gate math: psA [128,16]=rz pre (+bias+gi), psB [128,8]=ghn (+bhh_n),
    gin_ap = gi_n (+b_ih_n) [128,8] in SBUF, hprev_ap [128,8] bf16.
    Writes h_new (bf16) to hout_ap."""
    rr = work.tile([128, 8], f32, tag=f"rr{tag}")
    nc.scalar.activation(rr, psA[:, 0:8], AF.Sigmoid)
    zt = work.tile([128, 8], f32, tag=f"zt{tag}")
    nc.scalar.activation(zt, psA[:, 8:16], AF.Sigmoid)
    t1 = work.tile([128, 8], bf16, tag=f"t1{tag}")
    nc.vector.tensor_tensor(t1, psB[:, 0:8], rr[:], ALU.mult)
    npre = work.tile([128, 8], bf16, tag=f"np{tag}")
    npre_i = nc.vector.tensor_tensor(npre, gin_ap, t1, ALU.add)
    n_ = work.tile([128, 8], bf16, tag=f"n{tag}")
    nc.scalar.activation(n_, npre, AF.Tanh)
    # zc/zh on DVE but explicitly ordered after npre (keep npre on the
    # critical path front of the DVE queue)
    zc = work.tile([128, 8], bf16, tag=f"zc{tag}")
    zc_i = nc.vector.tensor_scalar(zc, zt[:], -1.0, 1.0, ALU.mult, ALU.add)
    add_dep_helper(zc_i.ins, npre_i.ins, sync=False, reason="npre first")
    zh = work.tile([128, 8], bf16, tag=f"zh{tag}")
    zh_i = nc.vector.tensor_tensor(zh, hprev_ap, zt[:], ALU.mult)
    add_dep_helper(zh_i.ins, zc_i.ins, sync=False, reason="zc first")
    t2 = work.tile([128, 8], bf16, tag=f"t2{tag}")
    nc.vector.tensor_tensor(t2, n_, zc, ALU.mult)
    # h = t2 + zh materialized lazily (off the critical path); the next
    # scan step contracts t2 and zh directly via two accumulating passes
    nc.vector.tensor_tensor(hout_ap, t2, zh, ALU.add)
    return t2, zh


def _emit_gi_precompute(nc, psum_gi, gi32, wih, xT, gib, bhhn, n_steps):
    """gi32[:, 32t + {0:16 rz | 16:24 bhhn | 24:32 gin}] (bf16), biases folded."""
    # broadcast-fill the bhhn region once
    dst = _ap(gi32, 16, [[32, n_steps], [1, 8]])
    src = _ap(bhhn, 0, [[0, n_steps], [1, 8]])
    nc.vector.tensor_copy(dst, src)
    nchunks = (n_steps + 127) // 128
    for mt in range(NMT):
        off = 4 * mt if mt < 4 else 24 + 4 * (mt - 4)
        for ch in range(nchunks):
            t0 = ch * 128
            cnt = min(128, n_steps - t0)
            ps = psum_gi.tile([128, 512], f32, tag="gi")
            nc.tensor.matmul(ps[:, :cnt * 4], wih[:, mt * 128:(mt + 1) * 128],
                             xT[:, t0 * 4:(t0 + cnt) * 4], start=True, stop=True)
            dst = _ap(gi32, 32 * t0 + off, [[32, cnt], [1, 4]])
            src = ps[:, :cnt * 4].rearrange("p (t b) -> p t b", b=4)
            gcol = gib[:, mt:mt + 1]
            if mt % 2 == 0:
                nc.vector.tensor_scalar(dst, src, gcol, None, ALU.add)
            else:
                nc.scalar.activation(dst, src, AF.Identity, bias=gcol)


def _emit_scan(nc, psum_gh, work, h0, hfull, hkh, hout, gi32, whh, ident,
               n_steps, tag):
    """h0: [128,8] initial state tile. hfull(t)->[128,(kh,b)] view of h_t;
    hkh(t,kh)->[128,4] rhs view; hout(t)->write view."""
    t2p = zhp = None
    for t in range(n_steps):
        psA = psum_gh.tile([128, 16], f32, tag="ghA")
        psB = psum_gh.tile([128, 8], f32, tag="ghB")
        nc.tensor.matmul(psA[:, 0:16], ident, gi32[:, 32 * t:32 * t + 16],
                         start=True, stop=False, skip_group_check=True)
        nc.tensor.matmul(psB[:, 0:8], ident, gi32[:, 32 * t + 16:32 * t + 24],
                         start=True, stop=False, skip_group_check=True)

        def mm_pass(rhs_t, stop_last):
            for mt in (0, 1, 4, 5, 2, 3):
                for kh in range(KH):
                    out = (psA[:, 4 * mt:4 * mt + 4] if mt < 4
                           else psB[:, 4 * (mt - 4):4 * (mt - 4) + 4])
                    nc.tensor.matmul(
                        out,
                        whh[:, (kh * NMT + mt) * 128:(kh * NMT + mt + 1) * 128],
                        rhs_t[:, 4 * kh:4 * kh + 4],
                        start=False,
                        stop=(stop_last and mt == NMT - 1 and kh == KH - 1),
                        skip_group_check=True)

        if t == 0:
            mm_pass(h0, True)
        else:
            # gh = Whh*(t2+zh): zh pass runs early (hidden under tanh),
            # only the t2 pass is on the critical path
            mm_pass(zhp, False)
            mm_pass(t2p, True)
        hprev = h0[:] if t == 0 else hfull(t - 1)
        t2p, zhp = _emit_gru_cell_gates(nc, work, psA, psB,
                                        gi32[:, 32 * t + 24:32 * t + 32],
                                        hprev, hout(t), tag)


def build_nc(S=S_FULL, SQ=SQ_FULL, num_devices=NCORES, dbg=False):
    nc = bacc.Bacc("TRN2", target_bir_lowering=False, debug=False,
                   enable_asserts=True, num_devices=num_devices)
    io = {"_dbg": dbg}

    def inp(name, shape, dtype):
        io[name] = nc.dram_tensor(name, shape, dtype, kind="ExternalInput").ap()

    def outp(name, shape, dtype):
        io[name] = nc.dram_tensor(name, shape, dtype, kind="ExternalOutput").ap()

    inp("cT", [128, S * BL], bf16)
    inp("qT", [128, SQ * BL], bf16)
    inp("h0c", [128, KH * BL], bf16)
    inp("h0q", [128, KH * BL], bf16)
    for p in ("c", "q"):
        inp(f"wih_{p}", [128, NMT * 128], bf16)
        inp(f"whh_{p}", [128, KH * NMT * 128], bf16)
        inp(f"gib_{p}", [128, NMT], f32)
        inp(f"bhhn_{p}", [128, 8], bf16)
    inp("ident", [128, 128], bf16)
    inp("gidx", [128, 32], i16)
    inp("maskadd", [1, BL * N], f32)
    inp("bonehot", [BL, BL * N], bf16)
    inp("ones_k1", [1, 128], f32)
    inp("ones_col", [128, 1], f32)
    inp("attw", [128, KH * KH * 128], bf16)
    inp("w1b", [128, 7 * KH * 100], bf16)      # blocks cs,m,qh,csq,csm,acsq,acsm
    inp("w1dq", [1, 100], bf16)
    inp("w1dm", [1, 100], bf16)
    inp("b1", [A, 1], f32)
    inp("w2", [A, 1], bf16)
    inp("b2", [1, 1], f32)
    inp("memw_ih", [128, KH * NMT * 128], bf16)
    inp("memw_hh", [128, KH * NMT * 128], bf16)
    inp("memb", [128, 32], bf16)
    inp("answ_ih", [128, 18 * NMT * 128], bf16)
    inp("answ_hh", [128, KH * NMT * 128], bf16)
    inp("ansb", [128, 32], bf16)
    inp("outw", [128, KH * OT * 128], bf16)
    inp("outb_bc", [128, OT * BL], bf16)
    outp("y_out", [BL, O], f32)
    outp("att_out", [MEM_COUNT, BL, N], f32)
    if dbg:
        outp("dbg_qh", [128, 8], f32)
        outp("dbg_hf", [128, 8], f32)
        outp("dbg_cs", [128, KH * BL * N], bf16)
        outp("dbg_h1", [A, BL * N], bf16)
        outp("dbg_e0", [128, 8], f32)
        outp("dbg_m1", [128, 8], f32)

    with tile.TileContext(nc) as tc:
        _emit(nc, tc, io, S, SQ)
    nc.compile()
    return nc, io


def _emit(nc, tc, io, S, SQ):
    with (
        tc.tile_pool(name="big", bufs=1) as big,
        tc.tile_pool(name="consts", bufs=1) as consts,
        tc.tile_pool(name="dram", bufs=1, space="DRAM") as dram,
        tc.tile_pool(name="work", bufs=3) as work,
    ):
        # ---- load constants / inputs to SBUF ----
        def load(name, shape=None, dtype=None, pool=consts):
            src = io[name]
            shape = shape or [src.ap[0][1]] + [d[1] for d in src.ap[1:]]
            dtype = dtype or src.dtype
            t = pool.tile(shape, dtype, tag=name)
            nc.sync.dma_start(out=t[:], in_=src)
            return t

        cT = load("cT", pool=big)
        qT = load("qT")
        wih_c = load("wih_c"); whh_c = load("whh_c", pool=big)
        gib_c = load("gib_c"); bhhn_c = load("bhhn_c")
        wih_q = load("wih_q"); whh_q = load("whh_q")
        gib_q = load("gib_q"); bhhn_q = load("bhhn_q")
        ident = load("ident")
        gidx = load("gidx")
        maskadd = load("maskadd")
        bonehot = load("bonehot")
        ones_k1 = load("ones_k1")
        ones_col = load("ones_col")
        attw = load("attw")
        w1b = load("w1b"); w1dq = load("w1dq"); w1dm = load("w1dm")
        b1 = load("b1"); w2 = load("w2"); b2 = load("b2")
        memw_ih = load("memw_ih"); memw_hh = load("memw_hh"); memb = load("memb")
        answ_ih = load("answ_ih", pool=big); answ_hh = load("answ_hh")
        ansb = load("ansb")
        outw = load("outw", pool=big); outb_bc = load("outb_bc")

        gic = big.tile([128, 32 * S], bf16, tag="gic")
        giq = big.tile([128, 32 * SQ], bf16, tag="giq")
        # context hist: block layout col = (t//16)*128 + kh*64 + b*16 + t%16
        assert S % 16 == 0
        nblocks = S // 16
        histc = big.tile([128, nblocks * 128], bf16, tag="histc")
        histq = big.tile([128, 8 * SQ], bf16, tag="histq")
        h0c = load("h0c")
        h0q = load("h0q")

        def c_base(t):
            return (t // 16) * 128 + (t % 16)

        def c_full(t):
            return _ap(histc[:], c_base(t), [[64, KH], [16, BL]])

        def c_kh(t, kh):
            return _ap(histc[:], c_base(t) + 64 * kh, [[16, BL]])

        def q_full(t):
            return histq[:, 8 * t:8 * t + 8]

        def q_kh(t, kh):
            return histq[:, 8 * t + 4 * kh:8 * t + 4 * kh + 4]

        # ---- GRU scans ----
        with (tc.tile_pool(name="psg", bufs=2, space="PSUM") as psum_gi,
              tc.tile_pool(name="psh", bufs=2, space="PSUM") as psum_gh):
            _emit_gi_precompute(nc, psum_gi, giq, wih_q, qT, gib_q, bhhn_q, SQ)
            _emit_scan(nc, psum_gh, work, h0q, q_full, q_kh, q_full, giq,
                       whh_q, ident, SQ, "q")
            _emit_gi_precompute(nc, psum_gi, gic, wih_c, cT, gib_c, bhhn_c, S)
            _emit_scan(nc, psum_gh, work, h0c, c_full, c_kh, c_full, gic,
                       whh_c, ident, S, "c")

        qh = consts.tile([128, 8], bf16, tag="qh")
        nc.vector.tensor_copy(qh[:], q_full(SQ - 1))
        if io["_dbg"]:
            dqh = work.tile([128, 8], f32, tag="dqh")
            nc.vector.tensor_copy(dqh[:], q_full(SQ - 1))
            nc.sync.dma_start(out=io["dbg_qh"], in_=dqh[:])
            dhf = work.tile([128, 8], f32, tag="dhf")
            nc.vector.tensor_copy(dhf[:], c_full(S - 1))
            nc.sync.dma_start(out=io["dbg_hf"], in_=dhf[:])

        # ---- dump context hidden history to DRAM, gather sentence ends ----
        # ---- transpose hist blocks to (kh,b,t16)-on-partition, dump to DRAM ----
        hD = dram.tile([BL * S, 256], bf16, tag="hD")
        histT2 = big.tile([128, nblocks * 128], bf16, tag="histT2")
        with tc.tile_pool(name="pst", bufs=2, space="PSUM") as pst:
            for blk in range(nblocks):
                pt = pst.tile([128, 128], bf16, tag="pt")
                nc.tensor.transpose(pt, histc[:, blk * 128:(blk + 1) * 128],
                                    ident[:])
                if blk % 2 == 0:
                    nc.vector.tensor_copy(histT2[:, blk * 128:(blk + 1) * 128], pt)
                else:
                    nc.scalar.copy(histT2[:, blk * 128:(blk + 1) * 128], pt)
        for kh in range(KH):
            for b in range(BL):
                src = _app(histT2[:], kh * 64 + b * 16, 16, 0,
                           [[128, nblocks], [1, 128]])
                dst = bass.AP(tensor=hD[:].tensor,
                              offset=hD[:].offset + b * S * 256 + kh * 128,
                              ap=[[256, 16], [4096, nblocks], [1, 128]])
                nc.sync.dma_start(out=dst, in_=src)

        cs = big.tile([128, KH, BL * N], bf16, tag="cs")
        nc.gpsimd.dma_gather(out_ap=cs[:], in_ap=hD[:], idxs_ap=gidx[:],
                             num_idxs=BL * N, num_idxs_reg=BL * N,
                             elem_size=256, transpose=True)
        if io["_dbg"]:
            nc.sync.dma_start(out=io["dbg_cs"],
                              in_=cs[:].rearrange("p a b -> p (a b)"))

        # ---- attention precompute (hop-invariant) ----
        with (tc.tile_pool(name="att", bufs=1) as att,
              tc.tile_pool(name="attps", bufs=3, space="PSUM") as attps):
            # cw = att_weight.T @ c_selT   [128, KH, 512]
            cw = att.tile([128, KH, BL * N], bf16, tag="cw")
            for mh in range(KH):
                pcw = attps.tile([128, BL * N], f32, tag="ps")
                for kh in range(KH):
                    nc.tensor.matmul(
                        pcw, attw[:, (kh * KH + mh) * 128:(kh * KH + mh + 1) * 128],
                        cs[:, kh, :], start=(kh == 0), stop=(kh == KH - 1))
                nc.vector.tensor_copy(cw[:, mh, :], pcw)

            def fold_vec(vT, blk, tag):
                # (w1[:, blk] @ v) -> [BL, A] bf16 (lhsT for b-onehot matmul)
                pf = attps.tile([BL, A], f32, tag="ps")
                for kh in range(KH):
                    nc.tensor.matmul(pf, vT[:, 4 * kh:4 * kh + 4],
                                     w1b[:, (blk * KH + kh) * A:(blk * KH + kh + 1) * A],
                                     start=(kh == 0), stop=(kh == KH - 1))
                sb = att.tile([BL, A], bf16, tag=tag)
                nc.vector.tensor_copy(sb[:], pf)
                return sb

            qhf = fold_vec(qh, 2, "qhf")

            def dots(vT, dst):
                # dst[0, (b,n)] = sum_h cw[h,(b,n)] * v[h,b]
                pd = attps.tile([1, BL * N], f32, tag="ps")
                for b in range(BL):
                    for kh in range(KH):
                        nc.tensor.matmul(
                            pd[:, b * N:(b + 1) * N],
                            vT[:, 4 * kh + b:4 * kh + b + 1],
                            cw[:, kh, b * N:(b + 1) * N],
                            start=(kh == 0), stop=(kh == KH - 1))
                nc.vector.tensor_copy(dst[:], pd)

            dq_sb = att.tile([1, BL * N], bf16, tag="dq_sb")
            dm_sb = att.tile([1, BL * N], bf16, tag="dm_sb")
            dots(qh, dq_sb)

            def bcast_kh(vT):
                # [128, 8] -> virtual [128, KH, BL, N] (stride-0 over n)
                return _ap(vT[:], 0, [[4, KH], [1, BL], [0, N]])

            csv = cs[:].rearrange("p kh (b n) -> p kh b n", b=BL)
            csq = att.tile([128, KH, BL, N], bf16, tag="csq")
            nc.vector.tensor_tensor(csq, csv, bcast_kh(qh), ALU.mult)
            dq_ = att.tile([128, KH, BL, N], bf16, tag="dq_")
            nc.vector.tensor_tensor(dq_, csv, bcast_kh(qh), ALU.subtract)
            acsq = att.tile([128, KH, BL, N], bf16, tag="acsq")
            nc.vector.scalar_tensor_tensor(acsq, dq_, -1.0, dq_, ALU.mult, ALU.max)

            m_cur = qh
            # ---- hops ----
            for hop in range(MEM_COUNT):
                mf = fold_vec(m_cur, 1, "mf")
                dots(m_cur, dm_sb)
                csm = att.tile([128, KH, BL, N], bf16, tag="csm")
                nc.vector.tensor_tensor(csm, csv, bcast_kh(m_cur), ALU.mult)
                dm_ = att.tile([128, KH, BL, N], bf16, tag="dm_")
                nc.vector.tensor_tensor(dm_, csv, bcast_kh(m_cur), ALU.subtract)
                acsm = att.tile([128, KH, BL, N], bf16, tag="acsm")
                nc.vector.scalar_tensor_tensor(acsm, dm_, -1.0, dm_, ALU.mult, ALU.max)

                # h1 = tanh(feats @ w1.T + b1): accumulate [A, 512]
                ph1 = attps.tile([A, BL * N], f32, tag="ps")
                first = True
                for blk, src_t in ((0, cs), (3, csq), (4, csm), (5, acsq), (6, acsm)):
                    for kh in range(KH):
                        nc.tensor.matmul(
                            ph1, w1b[:, (blk * KH + kh) * A:(blk * KH + kh + 1) * A],
                            src_t[:, kh].rearrange("p b n -> p (b n)") if src_t is not cs
                            else cs[:, kh, :],
                            start=first, stop=False, skip_group_check=True)
                        first = False
                nc.tensor.matmul(ph1, w1dq[:], dq_sb[:], start=False, stop=False,
                                 skip_group_check=True)
                nc.tensor.matmul(ph1, w1dm[:], dm_sb[:], start=False, stop=False,
                                 skip_group_check=True)
                nc.tensor.matmul(ph1, qhf[:], bonehot[:], start=False, stop=False,
                                 skip_group_check=True)
                nc.tensor.matmul(ph1, mf[:], bonehot[:], start=False, stop=True,
                                 skip_group_check=True)
                h1s = att.tile([A, BL * N], bf16, tag="h1s")
                nc.scalar.activation(h1s, ph1, AF.Tanh, bias=b1[:])
                if io["_dbg"] and hop == 0:
                    nc.sync.dma_start(out=io["dbg_h1"], in_=h1s[:])

                # scores -> masked softmax over n
                psc = attps.tile([1, BL * N], f32, tag="ps")
                nc.tensor.matmul(psc, w2[:], h1s[:], start=True, stop=True)
                sc = att.tile([1, BL * N], f32, tag="sc")
                nc.vector.scalar_tensor_tensor(sc, psc, b2[:, 0:1], maskadd[:],
                                               ALU.add, ALU.add)
                pe = att.tile([1, BL * N], f32, tag="pe")
                nc.scalar.activation(pe, sc, AF.Exp)
                ssum = att.tile([1, BL], f32, tag="ssum")
                nc.vector.tensor_reduce(
                    ssum, pe[:].rearrange("p (b n) -> p b n", b=BL),
                    axis=mybir.AxisListType.X, op=ALU.add)
                srec = att.tile([1, BL], f32, tag="srec")
                nc.vector.reciprocal(srec, ssum)
                p_ = att.tile([1, BL * N], f32, tag="p_")
                nc.vector.tensor_tensor(
                    p_.rearrange("p (b n) -> p b n", b=BL),
                    pe[:].rearrange("p (b n) -> p b n", b=BL),
                    _ap(srec[:], 0, [[1, BL], [0, N]]), ALU.mult)
                dstp = bass.AP(tensor=io["att_out"].tensor,
                               offset=io["att_out"].offset + hop * BL * N,
                               ap=[[BL * N, 1], [1, BL * N]])
                nc.sync.dma_start(out=dstp, in_=p_[:])

                # e = sum_n p * c_sel  -> [128, 8] bf16
                ppb = attps.tile([128, BL * N], f32, tag="ps")
                nc.tensor.matmul(ppb, ones_k1[:], p_[:], start=True, stop=True)
                pbs = att.tile([128, BL * N], bf16, tag="pbs")
                nc.vector.tensor_copy(pbs[:], ppb)
                tmpe = att.tile([128, KH, BL, N], f32, tag="tmpe")
                nc.vector.tensor_tensor(
                    tmpe, csv, _ap(pbs[:], 0, [[0, KH], [N, BL], [1, N]]), ALU.mult)
                ef = att.tile([128, KH, BL], f32, tag="ef")
                nc.vector.tensor_reduce(ef, tmpe[:], axis=mybir.AxisListType.X,
                                        op=ALU.add)
                eT = att.tile([128, 8], bf16, tag="eT")
                nc.vector.tensor_copy(eT[:], ef[:].rearrange("p kh b -> p (kh b)"))
                if io["_dbg"] and hop == 0:
                    de0 = att.tile([128, 8], f32, tag="de0")
                    nc.vector.tensor_copy(de0[:], eT[:])
                    nc.sync.dma_start(out=io["dbg_e0"], in_=de0[:])

                # mem GRU cell: m_new = GRUCell(e, m)
                pm = attps.tile([128, 32], f32, tag="ps")
                nc.tensor.matmul(pm[:, 0:32], ident[:], memb[:],
                                 start=True, stop=False, skip_group_check=True)
                for mt in range(NMT):
                    col = 4 * mt if mt < 4 else 16 + 4 * (mt - 4)
                    for kh in range(KH):
                        nc.tensor.matmul(
                            pm[:, col:col + 4],
                            memw_ih[:, (kh * NMT + mt) * 128:(kh * NMT + mt + 1) * 128],
                            eT[:, 4 * kh:4 * kh + 4],
                            start=False, stop=False, skip_group_check=True)
                for mt in range(NMT):
                    col = 4 * mt if mt < 4 else 24 + 4 * (mt - 4)
                    for kh in range(KH):
                        nc.tensor.matmul(
                            pm[:, col:col + 4],
                            memw_hh[:, (kh * NMT + mt) * 128:(kh * NMT + mt + 1) * 128],
                            m_cur[:, 4 * kh:4 * kh + 4],
                            start=False, stop=(mt == NMT - 1 and kh == KH - 1),
                            skip_group_check=True)
                m_new = att.tile([128, 8], bf16, tag=f"m{hop}")
                _emit_gru_cell_gates_mem(nc, work, pm, m_cur, m_new, f"m{hop}")
                if io["_dbg"] and hop == 0:
                    dm1 = att.tile([128, 8], f32, tag="dm1")
                    nc.vector.tensor_copy(dm1[:], m_new[:])
                    nc.sync.dma_start(out=io["dbg_m1"], in_=dm1[:])
                m_cur = m_new

            # ---- answer module ----
            msq = m_cur
            for it in range(ANS_COUNT + 1):
                py = attps.tile([128, OT * BL], f32, tag="ps")
                nc.tensor.matmul(py[:, 0:OT * BL], ident[:], outb_bc[:],
                                 start=True, stop=False, skip_group_check=True)
                for mo in range(OT):
                    for kh in range(KH):
                        nc.tensor.matmul(
                            py[:, mo * BL:(mo + 1) * BL],
                            outw[:, (kh * OT + mo) * 128:(kh * OT + mo + 1) * 128],
                            msq[:, 4 * kh:4 * kh + 4],
                            start=False, stop=(kh == KH - 1), skip_group_check=True)
                yexp = att.tile([128, OT, BL], f32, tag="yexp")
                nc.scalar.activation(yexp, py[:].rearrange("p (mo b) -> p mo b", b=BL),
                                     AF.Exp)
                rsum = att.tile([128, BL], f32, tag="rsum")
                nc.vector.tensor_reduce(
                    rsum, _ap(yexp[:], 0, [[1, BL], [BL, OT]]),
                    axis=mybir.AxisListType.X, op=ALU.add)
                pcs = attps.tile([1, BL], f32, tag="ps")
                nc.tensor.matmul(pcs, ones_col[:], rsum[:], start=True, stop=True)
                yrec = att.tile([1, BL], f32, tag="yrec")
                nc.vector.reciprocal(yrec, pcs)
                prb = attps.tile([128, BL], f32, tag="ps")
                nc.tensor.matmul(prb, ones_k1[:], yrec[:], start=True, stop=True)
                if it == ANS_COUNT:
                    yf = att.tile([128, OT, BL], f32, tag="yf")
                    nc.vector.tensor_tensor(yf, yexp[:],
                                            _ap(prb[:], 0, [[0, OT], [1, BL]]),
                                            ALU.mult)
                    for b in range(BL):
                        for mo in range(OT):
                            rows = 128 if mo < OT - 1 else OT_LAST
                            dsty = bass.AP(
                                tensor=io["y_out"].tensor,
                                offset=io["y_out"].offset + b * O + mo * 128,
                                ap=[[1, rows], [1, 1]])
                            nc.sync.dma_start(
                                out=dsty,
                                in_=_app(yf[:], 0, rows, mo * BL + b, [[1, 1]]))
                else:
                    yn = att.tile([128, OT, BL], bf16, tag="yn")
                    nc.vector.tensor_tensor(yn, yexp[:],
                                            _ap(prb[:], 0, [[0, OT], [1, BL]]),
                                            ALU.mult)
                    pa = attps.tile([128, 32], f32, tag="ps")
                    nc.tensor.matmul(pa[:, 0:32], ident[:], ansb[:],
                                     start=True, stop=False, skip_group_check=True)
                    for mt in range(NMT):
                        col = 4 * mt if mt < 4 else 16 + 4 * (mt - 4)
                        for kt in range(16):
                            nc.tensor.matmul(
                                pa[:, col:col + 4],
                                answ_ih[:, (kt * NMT + mt) * 128:(kt * NMT + mt + 1) * 128],
                                yn[:, kt, :],
                                start=False, stop=False, skip_group_check=True)
                        for kh in range(KH):
                            nc.tensor.matmul(
                                pa[:, col:col + 4],
                                answ_ih[:, ((16 + kh) * NMT + mt) * 128:((16 + kh) * NMT + mt + 1) * 128],
                                qh[:, 4 * kh:4 * kh + 4],
                                start=False, stop=False, skip_group_check=True)
                    for mt in range(NMT):
                        col = 4 * mt if mt < 4 else 24 + 4 * (mt - 4)
                        for kh in range(KH):
                            nc.tensor.matmul(
                                pa[:, col:col + 4],
                                answ_hh[:, (kh * NMT + mt) * 128:(kh * NMT + mt + 1) * 128],
                                msq[:, 4 * kh:4 * kh + 4],
                                start=False, stop=(mt == NMT - 1 and kh == KH - 1),
                                skip_group_check=True)
                    msq_new = att.tile([128, 8], bf16, tag=f"msq{it}")
                    _emit_gru_cell_gates_mem(nc, work, pa, msq, msq_new, f"a{it}")
                    msq = msq_new


def _emit_gru_cell_gates_mem(nc, work, ps, hprev, hout, tag):
    """Gate math for psum layout [0:16 rz | 16:24 gin | 24:32 ghn]."""
    rz = work.tile([128, 16], f32, tag=f"rz{tag}")
    nc.scalar.activation(rz, ps[:, 0:16], AF.Sigmoid)
    zc = work.tile([128, 8], f32, tag=f"zc{tag}")
    nc.scalar.activation(zc, ps[:, 8:16], AF.Sigmoid, scale=-1.0)
    t1 = work.tile([128, 8], bf16, tag=f"t1{tag}")
    nc.vector.tensor_tensor(t1, ps[:, 24:32], rz[:, 0:8], ALU.mult)
    npre = work.tile([128, 8], bf16, tag=f"np{tag}")
    nc.vector.tensor_tensor(npre, ps[:, 16:24], t1, ALU.add)
    n_ = work.tile([128, 8], f32, tag=f"n{tag}")
    nc.scalar.activation(n_, npre, AF.Tanh)
    zh = work.tile([128, 8], f32, tag=f"zh{tag}")
    nc.vector.tensor_tensor(zh, hprev[:], rz[:, 8:16], ALU.mult)
    t2 = work.tile([128, 8], f32, tag=f"t2{tag}")
    nc.vector.tensor_tensor(t2, n_, zc, ALU.mult)
    nc.vector.tensor_tensor(hout[:], t2, zh, ALU.add)


# ---------------------------------------------------------------------------
# host-side prep
# ---------------------------------------------------------------------------

def _gate_tiles_ih(w):  # w [768, K] -> [128(K... per-tile k), NMT*128]
    K = w.shape[1]
    assert K == 128
    out = np.zeros((128, NMT * 128), np.float32)
    for mt in range(NMT):
        out[:, mt * 128:(mt + 1) * 128] = w[mt * 128:(mt + 1) * 128, :].T
    return out


def _gate_tiles_k(w, nkt=None):  # w [768, K], K = nkt*128 (zero-pad) -> [128, nkt*NMT*128]
    K = w.shape[1]
    nkt = nkt or (K + 127) // 128
    wp = np.zeros((G3, nkt * 128), np.float32)
    wp[:, :K] = w
    out = np.zeros((128, nkt * NMT * 128), np.float32)
    for kt in range(nkt):
        for mt in range(NMT):
            out[:, (kt * NMT + mt) * 128:(kt * NMT + mt + 1) * 128] = \
                wp[mt * 128:(mt + 1) * 128, kt * 128:(kt + 1) * 128].T
    return out


def _gru_consts(pfx, b_ih, b_hh):
    gib = np.zeros((128, NMT), np.float32)
    for mt in range(NMT):
        gib[:, mt] = b_ih[mt * 128:(mt + 1) * 128]
        if mt < 4:
            gib[:, mt] += b_hh[mt * 128:(mt + 1) * 128]
    bhhn = np.zeros((128, 8), np.float32)
    for i in range(2):
        for b in range(BL):
            bhhn[:, i * 4 + b] = b_hh[(4 + i) * 128:(5 + i) * 128]
    return {f"gib_{pfx}": gib, f"bhhn_{pfx}": _bf(bhhn)}


def _cell_bias(b_ih, b_hh):
    out = np.zeros((128, 32), np.float32)
    for mt in range(4):
        for b in range(BL):
            out[:, mt * 4 + b] = (b_ih + b_hh)[mt * 128:(mt + 1) * 128]
    for i in range(2):
        for b in range(BL):
            out[:, 16 + i * 4 + b] = b_ih[(4 + i) * 128:(5 + i) * 128]
            out[:, 24 + i * 4 + b] = b_hh[(4 + i) * 128:(5 + i) * 128]
    return out


def _bf(x):
    import ml_dtypes
    return np.asarray(x, np.float32).astype(ml_dtypes.bfloat16)


def prep_inputs(inputs, S=S_FULL, SQ=SQ_FULL):
    """Returns (in_maps list per core)."""
    ii = {k: np.asarray(v) for k, v in inputs.items()}
    shared = {}
    shared["wih_c"] = _bf(_gate_tiles_ih(ii["in_w_ih"]))
    shared["whh_c"] = _bf(_gate_tiles_k(ii["in_w_hh"]))
    shared["wih_q"] = _bf(_gate_tiles_ih(ii["qe_w_ih"]))
    shared["whh_q"] = _bf(_gate_tiles_k(ii["qe_w_hh"]))
    shared.update(_gru_consts("c", ii["in_b_ih"], ii["in_b_hh"]))
    shared.update(_gru_consts("q", ii["qe_b_ih"], ii["qe_b_hh"]))
    shared["ident"] = _bf(np.eye(128, dtype=np.float32))
    shared["ones_k1"] = np.ones((1, 128), np.float32)
    shared["ones_col"] = np.ones((128, 1), np.float32)
    # att_weight [H, H] -> lhsT tiles [k, (kh*KH+mh)*128+m]
    aw = np.zeros((128, KH * KH * 128), np.float32)
    for kh in range(KH):
        for mh in range(KH):
            aw[:, (kh * KH + mh) * 128:(kh * KH + mh + 1) * 128] = \
                ii["att_weight"][kh * 128:(kh + 1) * 128, mh * 128:(mh + 1) * 128]
    shared["attw"] = _bf(aw)
    w1 = ii["att_w1"]  # [A, 7H+2]
    w1b = np.zeros((128, 7 * KH * A), np.float32)
    for blk in range(7):
        for kh in range(KH):
            w1b[:, (blk * KH + kh) * A:(blk * KH + kh + 1) * A] = \
                w1[:, blk * H + kh * 128:blk * H + (kh + 1) * 128].T
    shared["w1b"] = _bf(w1b)
    shared["w1dq"] = _bf(w1[:, 7 * H:7 * H + 1].T)
    shared["w1dm"] = _bf(w1[:, 7 * H + 1:7 * H + 2].T)
    shared["b1"] = np.asarray(ii["att_b1"], np.float32).reshape(A, 1)
    shared["w2"] = _bf(ii["att_w2"].reshape(1, A).T)
    shared["b2"] = np.asarray(ii["att_b2"], np.float32).reshape(1, 1)
    shared["memw_ih"] = _bf(_gate_tiles_k(ii["mem_w_ih"]))
    shared["memw_hh"] = _bf(_gate_tiles_k(ii["mem_w_hh"]))
    shared["memb"] = _bf(_cell_bias(ii["mem_b_ih"], ii["mem_b_hh"]))
    # ans_w_ih [768, O+H]: y part zero-padded to 16*128, then qh part
    awi = ii["ans_w_ih"]
    awi_p = np.zeros((G3, 18 * 128), np.float32)
    awi_p[:, :O] = awi[:, :O]
    awi_p[:, 16 * 128:16 * 128 + H] = awi[:, O:]
    shared["answ_ih"] = _bf(_gate_tiles_k(awi_p))
    shared["answ_hh"] = _bf(_gate_tiles_k(ii["ans_w_hh"]))
    shared["ansb"] = _bf(_cell_bias(ii["ans_b_ih"], ii["ans_b_hh"]))
    ow = np.zeros((128, KH * OT * 128), np.float32)
    owp = np.zeros((OT * 128, H), np.float32)
    owp[:O] = ii["out_w"]
    for kh in range(KH):
        for mo in range(OT):
            ow[:, (kh * OT + mo) * 128:(kh * OT + mo + 1) * 128] = \
                owp[mo * 128:(mo + 1) * 128, kh * 128:(kh + 1) * 128].T
    shared["outw"] = _bf(ow)
    ob = np.zeros((128, OT * BL), np.float32)
    obp = np.full(OT * 128, NEG, np.float32)
    obp[:O] = ii["out_b"]
    for mo in range(OT):
        for b in range(BL):
            ob[:, mo * BL + b] = obp[mo * 128:(mo + 1) * 128]
    shared["outb_bc"] = _bf(ob)

    c = np.asarray(ii["c"], np.float32)[:, :S]
    q = np.asarray(ii["q"], np.float32)[:, :SQ]
    cidx = np.asarray(ii["c_index"]).astype(np.int64)
    lc = np.asarray(ii["len_c"]).astype(np.int64)
    i_state = np.asarray(ii["i_state"], np.float32)
    q_state = np.asarray(ii["q_state"], np.float32)

    in_maps = []
    for core in range(NCORES):
        b0 = core * BL
        m = dict(shared)
        # cT[d, t*BL+b] = c[b0+b, t, d]
        m["cT"] = _bf(c[b0:b0 + BL].transpose(2, 1, 0).reshape(DIN, S * BL))
        m["qT"] = _bf(q[b0:b0 + BL].transpose(2, 1, 0).reshape(DIN, SQ * BL))
        # h0[p, kh*BL+b] = state[0, b0+b, kh*128+p]
        m["h0c"] = _bf(i_state[0, b0:b0 + BL].reshape(BL, KH, 128)
                       .transpose(2, 1, 0).reshape(128, KH * BL))
        m["h0q"] = _bf(q_state[0, b0:b0 + BL].reshape(BL, KH, 128)
                       .transpose(2, 1, 0).reshape(128, KH * BL))
        # gather rows: row index b*S + c_index[b0+b, n], order k=b*N+n
        rows = (np.arange(BL)[:, None] * S + cidx[b0:b0 + BL]).reshape(-1)
        gw = np.zeros((16, 32), np.int16)
        for k in range(BL * N):
            gw[k % 16, k // 16] = rows[k]
        m["gidx"] = np.tile(gw, (8, 1))
        mask = np.zeros((1, BL * N), np.float32)
        for b in range(BL):
            mask[0, b * N:(b + 1) * N] = np.where(np.arange(N) < lc[b0 + b], 0.0, NEG)
        m["maskadd"] = mask
        bo = np.zeros((BL, BL * N), np.float32)
        for b in range(BL):
            bo[b, b * N:(b + 1) * N] = 1.0
        m["bonehot"] = _bf(bo)
        in_maps.append(m)
    return in_maps


_CACHE = {}


def kernel(**inputs):
    key = "full"
    if key not in _CACHE:
        _CACHE[key] = build_nc()
    nc, _ = _CACHE[key]
    in_maps = prep_inputs(inputs)
    res = None
    for attempt in range(3):
        try:
            res = run_bass_kernel_spmd(nc, in_maps, list(range(NCORES)))
            break
        except Exception:
            if attempt == 2:
                raise
            import time as _time
            _time.sleep(25)
    y = np.concatenate([res.results[i]["y_out"] for i in range(NCORES)], axis=0)
    att = np.concatenate([res.results[i]["att_out"] for i in range(NCORES)], axis=1)
    return np.asarray(y, np.float32), np.asarray(att, np.float32)[..., None]


# revision 21
# speedup vs baseline: 1.0466x; 1.0466x over previous
"""Trainium2 Bass kernel for nn_DMN (Dynamic Memory Network).

Data-parallel over batch B=32 across 8 NeuronCores (4 rows/core).
Per core: fully-unrolled GRU scans (context S=1024, question SQ=32) in a
gates-on-partition layout ([6 gate-tiles of 128] x [4 batch]), input
projections precomputed as one big matmul, sentence-end gather via
DRAM round-trip + dma_gather(transpose=True), 3 attention hops, and the
answer module. Weights/activations in bf16, accumulation in fp32 PSUM.
"""

import sys

sys.path.insert(0, "/opt/trn_rl_repo")

import numpy as np

import concourse.bacc as bacc
import concourse.bass as bass
import concourse.tile as tile
from concourse.tile import add_dep_helper
from concourse import mybir
from concourse.bass_utils import run_bass_kernel_spmd

f32 = mybir.dt.float32
bf16 = mybir.dt.bfloat16
i16 = mybir.dt.int16
AF = mybir.ActivationFunctionType
ALU = mybir.AluOpType

# model dims (hardcoded per spec)
B, S_FULL, SQ_FULL, N, DIN, H, A, O = 32, 1024, 32, 128, 128, 256, 100, 2000
MEM_COUNT, ANS_COUNT = 3, 2
NCORES = 8
BL = B // NCORES           # 4 batch rows per core
G3 = 3 * H                 # 768 gates
NMT = G3 // 128            # 6 gate M-tiles
KH = H // 128              # 2 hidden K-tiles
OT = 16                    # output M-tiles (2000 -> 16 x 128, last ragged 80)
OT_LAST = O - 15 * 128     # 80
NEG = -1.0e38


def _ap(base, off, dims):
    """View into tile `base` (an AP) at element offset `off` with explicit
    free dims [[step, count], ...]; keeps base's partition dim."""
    return bass.AP(tensor=base.tensor, offset=base.offset + off,
                   ap=[list(base.ap[0])] + [list(d) for d in dims])


def _app(base, poff, pcount, off, dims):
    """Same but overriding partition range [poff, poff+pcount)."""
    pstep = base.ap[0][0]
    return bass.AP(tensor=base.tensor, offset=base.offset + poff * pstep + off,
                   ap=[[pstep, pcount]] + [list(d) for d in dims])


def _emit_gru_cell_gates(nc, work, psA, psB, gin_ap, hprev_ap, hout_ap, tag):
    """Gate math: psA [128,16]=rz pre (+bias+gi), psB [128,8]=ghn (+bhh_n),
    gin_ap = gi_n (+b_ih_n) [128,8] in SBUF, hprev_ap [128,8] bf16.
    Writes h_new (bf16) to hout_ap."""
    rz = work.tile([128, 16], f32, tag=f"rz{tag}")
    nc.scalar.activation(rz, psA[:, 0:16], AF.Sigmoid)
    t1 = work.tile([128, 8], bf16, tag=f"t1{tag}")
    nc.vector.tensor_tensor(t1, psB[:, 0:8], rz[:, 0:8], ALU.mult)
    npre = work.tile([128, 8], bf16, tag=f"np{tag}")
    npre_i = nc.vector.tensor_tensor(npre, gin_ap, t1, ALU.add)
    n_ = work.tile([128, 8], bf16, tag=f"n{tag}")
    nc.scalar.activation(n_, npre, AF.Tanh)
    # zc/zh on DVE but explicitly ordered after npre (keep npre on the
    # critical path front of the DVE queue)
    zc = work.tile([128, 8], bf16, tag=f"zc{tag}")
    zc_i = nc.vector.tensor_scalar(zc, rz[:, 8:16], -1.0, 1.0, ALU.mult, ALU.add)
    add_dep_helper(zc_i.ins, npre_i.ins, sync=False, reason="npre first")
    zh = work.tile([128, 8], bf16, tag=f"zh{tag}")
    zh_i = nc.vector.tensor_tensor(zh, hprev_ap, rz[:, 8:16], ALU.mult)
    add_dep_helper(zh_i.ins, zc_i.ins, sync=False, reason="zc first")
    t2 = work.tile([128, 8], bf16, tag=f"t2{tag}")
    nc.vector.tensor_tensor(t2, n_, zc, ALU.mult)
    # h = t2 + zh materialized lazily (off the critical path); the next
    # scan step contracts t2 and zh directly via two accumulating passes
    nc.vector.tensor_tensor(hout_ap, t2, zh, ALU.add)
    return t2, zh


def _emit_gi_precompute(nc, psum_gi, gi32, wih, xT, gib, bhhn, n_steps):
    """gi32[:, 32t + {0:16 rz | 16:24 bhhn | 24:32 gin}] (bf16), biases folded."""
    # broadcast-fill the bhhn region once
    dst = _ap(gi32, 16, [[32, n_steps], [1, 8]])
    src = _ap(bhhn, 0, [[0, n_steps], [1, 8]])
    nc.vector.tensor_copy(dst, src)
    nchunks = (n_steps + 127) // 128
    for mt in range(NMT):
        off = 4 * mt if mt < 4 else 24 + 4 * (mt - 4)
        for ch in range(nchunks):
            t0 = ch * 128
            cnt = min(128, n_steps - t0)
            ps = psum_gi.tile([128, 512], f32, tag="gi")
            nc.tensor.matmul(ps[:, :cnt * 4], wih[:, mt * 128:(mt + 1) * 128],
                             xT[:, t0 * 4:(t0 + cnt) * 4], start=True, stop=True)
            dst = _ap(gi32, 32 * t0 + off, [[32, cnt], [1, 4]])
            src = ps[:, :cnt * 4].rearrange("p (t b) -> p t b", b=4)
            gcol = gib[:, mt:mt + 1]
            if mt % 2 == 0:
                nc.vector.tensor_scalar(dst, src, gcol, None, ALU.add)
            else:
                nc.scalar.activation(dst, src, AF.Identity, bias=gcol)


def _emit_scan(nc, psum_gh, work, h0, hfull, hkh, hout, gi32, whh, ident,
               n_steps, tag):
    """h0: [128,8] initial state tile. hfull(t)->[128,(kh,b)] view of h_t;
    hkh(t,kh)->[128,4] rhs view; hout(t)->write view."""
    t2p = zhp = None
    for t in range(n_steps):
        psA = psum_gh.tile([128, 16], f32, tag="ghA")
        psB = psum_gh.tile([128, 8], f32, tag="ghB")
        nc.tensor.matmul(psA[:, 0:16], ident, gi32[:, 32 * t:32 * t + 16],
                         start=True, stop=False, skip_group_check=True)
        nc.tensor.matmul(psB[:, 0:8], ident, gi32[:, 32 * t + 16:32 * t + 24],
                         start=True, stop=False, skip_group_check=True)

        def mm_pass(rhs_t, stop_last):
            for mt in range(NMT):
                for kh in range(KH):
                    out = (psA[:, 4 * mt:4 * mt + 4] if mt < 4
                           else psB[:, 4 * (mt - 4):4 * (mt - 4) + 4])
                    nc.tensor.matmul(
                        out,
                        whh[:, (kh * NMT + mt) * 128:(kh * NMT + mt + 1) * 128],
                        rhs_t[:, 4 * kh:4 * kh + 4],
                        start=False,
                        stop=(stop_last and mt == NMT - 1 and kh == KH - 1),
                        skip_group_check=True)

        if t == 0:
            mm_pass(h0, True)
        else:
            # gh = Whh*(t2+zh): zh pass runs early (hidden under tanh),
            # only the t2 pass is on the critical path
            mm_pass(zhp, False)
            mm_pass(t2p, True)
        hprev = h0[:] if t == 0 else hfull(t - 1)
        t2p, zhp = _emit_gru_cell_gates(nc, work, psA, psB,
                                        gi32[:, 32 * t + 24:32 * t + 32],
                                        hprev, hout(t), tag)


def build_nc(S=S_FULL, SQ=SQ_FULL, num_devices=NCORES, dbg=False):
    nc = bacc.Bacc("TRN2", target_bir_lowering=False, debug=False,
                   enable_asserts=True, num_devices=num_devices)
    io = {"_dbg": dbg}

    def inp(name, shape, dtype):
        io[name] = nc.dram_tensor(name, shape, dtype, kind="ExternalInput").ap()

    def outp(name, shape, dtype):
        io[name] = nc.dram_tensor(name, shape, dtype, kind="ExternalOutput").ap()

    inp("cT", [128, S * BL], bf16)
    inp("qT", [128, SQ * BL], bf16)
    inp("h0c", [128, KH * BL], bf16)
    inp("h0q", [128, KH * BL], bf16)
    for p in ("c", "q"):
        inp(f"wih_{p}", [128, NMT * 128], bf16)
        inp(f"whh_{p}", [128, KH * NMT * 128], bf16)
        inp(f"gib_{p}", [128, NMT], f32)
        inp(f"bhhn_{p}", [128, 8], bf16)
    inp("ident", [128, 128], bf16)
    inp("gidx", [128, 32], i16)
    inp("maskadd", [1, BL * N], f32)
    inp("bonehot", [BL, BL * N], bf16)
    inp("ones_k1", [1, 128], f32)
    inp("ones_col", [128, 1], f32)
    inp("attw", [128, KH * KH * 128], bf16)
    inp("w1b", [128, 7 * KH * 100], bf16)      # blocks cs,m,qh,csq,csm,acsq,acsm
    inp("w1dq", [1, 100], bf16)
    inp("w1dm", [1, 100], bf16)
    inp("b1", [A, 1], f32)
    inp("w2", [A, 1], bf16)
    inp("b2", [1, 1], f32)
    inp("memw_ih", [128, KH * NMT * 128], bf16)
    inp("memw_hh", [128, KH * NMT * 128], bf16)
    inp("memb", [128, 32], bf16)
    inp("answ_ih", [128, 18 * NMT * 128], bf16)
    inp("answ_hh", [128, KH * NMT * 128], bf16)
    inp("ansb", [128, 32], bf16)
    inp("outw", [128, KH * OT * 128], bf16)
    inp("outb_bc", [128, OT * BL], bf16)
    outp("y_out", [BL, O], f32)
    outp("att_out", [MEM_COUNT, BL, N], f32)
    if dbg:
        outp("dbg_qh", [128, 8], f32)
        outp("dbg_hf", [128, 8], f32)
        outp("dbg_cs", [128, KH * BL * N], bf16)
        outp("dbg_h1", [A, BL * N], bf16)
        outp("dbg_e0", [128, 8], f32)
        outp("dbg_m1", [128, 8], f32)

    with tile.TileContext(nc) as tc:
        _emit(nc, tc, io, S, SQ)
    nc.compile()
    return nc, io


def _emit(nc, tc, io, S, SQ):
    with (
        tc.tile_pool(name="big", bufs=1) as big,
        tc.tile_pool(name="consts", bufs=1) as consts,
        tc.tile_pool(name="dram", bufs=1, space="DRAM") as dram,
        tc.tile_pool(name="work", bufs=3) as work,
    ):
        # ---- load constants / inputs to SBUF ----
        def load(name, shape=None, dtype=None, pool=consts):
            src = io[name]
            shape = shape or [src.ap[0][1]] + [d[1] for d in src.ap[1:]]
            dtype = dtype or src.dtype
            t = pool.tile(shape, dtype, tag=name)
            nc.sync.dma_start(out=t[:], in_=src)
            return t

        cT = load("cT", pool=big)
        qT = load("qT")
        wih_c = load("wih_c"); whh_c = load("whh_c", pool=big)
        gib_c = load("gib_c"); bhhn_c = load("bhhn_c")
        wih_q = load("wih_q"); whh_q = load("whh_q")
        gib_q = load("gib_q"); bhhn_q = load("bhhn_q")
        ident = load("ident")
        gidx = load("gidx")
        maskadd = load("maskadd")
        bonehot = load("bonehot")
        ones_k1 = load("ones_k1")
        ones_col = load("ones_col")
        attw = load("attw")
        w1b = load("w1b"); w1dq = load("w1dq"); w1dm = load("w1dm")
        b1 = load("b1"); w2 = load("w2"); b2 = load("b2")
        memw_ih = load("memw_ih"); memw_hh = load("memw_hh"); memb = load("memb")
        answ_ih = load("answ_ih", pool=big); answ_hh = load("answ_hh")
        ansb = load("ansb")
        outw = load("outw", pool=big); outb_bc = load("outb_bc")

        gic = big.tile([128, 32 * S], bf16, tag="gic")
        giq = big.tile([128, 32 * SQ], bf16, tag="giq")
        # context hist: block layout col = (t//16)*128 + kh*64 + b*16 + t%16
        assert S % 16 == 0
        nblocks = S // 16
        histc = big.tile([128, nblocks * 128], bf16, tag="histc")
        histq = big.tile([128, 8 * SQ], bf16, tag="histq")
        h0c = load("h0c")
        h0q = load("h0q")

        def c_base(t):
            return (t // 16) * 128 + (t % 16)

        def c_full(t):
            return _ap(histc[:], c_base(t), [[64, KH], [16, BL]])

        def c_kh(t, kh):
            return _ap(histc[:], c_base(t) + 64 * kh, [[16, BL]])

        def q_full(t):
            return histq[:, 8 * t:8 * t + 8]

        def q_kh(t, kh):
            return histq[:, 8 * t + 4 * kh:8 * t + 4 * kh + 4]

        # ---- GRU scans ----
        with (tc.tile_pool(name="psg", bufs=2, space="PSUM") as psum_gi,
              tc.tile_pool(name="psh", bufs=2, space="PSUM") as psum_gh):
            _emit_gi_precompute(nc, psum_gi, giq, wih_q, qT, gib_q, bhhn_q, SQ)
            _emit_scan(nc, psum_gh, work, h0q, q_full, q_kh, q_full, giq,
                       whh_q, ident, SQ, "q")
            _emit_gi_precompute(nc, psum_gi, gic, wih_c, cT, gib_c, bhhn_c, S)
            _emit_scan(nc, psum_gh, work, h0c, c_full, c_kh, c_full, gic,
                       whh_c, ident, S, "c")

        qh = consts.tile([128, 8], bf16, tag="qh")
        nc.vector.tensor_copy(qh[:], q_full(SQ - 1))
        if io["_dbg"]:
            dqh = work.tile([128, 8], f32, tag="dqh")
            nc.vector.tensor_copy(dqh[:], q_full(SQ - 1))
            nc.sync.dma_start(out=io["dbg_qh"], in_=dqh[:])
            dhf = work.tile([128, 8], f32, tag="dhf")
            nc.vector.tensor_copy(dhf[:], c_full(S - 1))
            nc.sync.dma_start(out=io["dbg_hf"], in_=dhf[:])

        # ---- dump context hidden history to DRAM, gather sentence ends ----
        # ---- transpose hist blocks to (kh,b,t16)-on-partition, dump to DRAM ----
        hD = dram.tile([BL * S, 256], bf16, tag="hD")
        histT2 = big.tile([128, nblocks * 128], bf16, tag="histT2")
        with tc.tile_pool(name="pst", bufs=2, space="PSUM") as pst:
            for blk in range(nblocks):
                pt = pst.tile([128, 128], bf16, tag="pt")
                nc.tensor.transpose(pt, histc[:, blk * 128:(blk + 1) * 128],
                                    ident[:])
                if blk % 2 == 0:
                    nc.vector.tensor_copy(histT2[:, blk * 128:(blk + 1) * 128], pt)
                else:
                    nc.scalar.copy(histT2[:, blk * 128:(blk + 1) * 128], pt)
        for kh in range(KH):
            for b in range(BL):
                src = _app(histT2[:], kh * 64 + b * 16, 16, 0,
                           [[128, nblocks], [1, 128]])
                dst = bass.AP(tensor=hD[:].tensor,
                              offset=hD[:].offset + b * S * 256 + kh * 128,
                              ap=[[256, 16], [4096, nblocks], [1, 128]])
                nc.sync.dma_start(out=dst, in_=src)

        cs = big.tile([128, KH, BL * N], bf16, tag="cs")
        nc.gpsimd.dma_gather(out_ap=cs[:], in_ap=hD[:], idxs_ap=gidx[:],
                             num_idxs=BL * N, num_idxs_reg=BL * N,
                             elem_size=256, transpose=True)
        if io["_dbg"]:
            nc.sync.dma_start(out=io["dbg_cs"],
                              in_=cs[:].rearrange("p a b -> p (a b)"))

        # ---- attention precompute (hop-invariant) ----
        with (tc.tile_pool(name="att", bufs=1) as att,
              tc.tile_pool(name="attps", bufs=3, space="PSUM") as attps):
            # cw = att_weight.T @ c_selT   [128, KH, 512]
            cw = att.tile([128, KH, BL * N], bf16, tag="cw")
            for mh in range(KH):
                pcw = attps.tile([128, BL * N], f32, tag="ps")
                for kh in range(KH):
                    nc.tensor.matmul(
                        pcw, attw[:, (kh * KH + mh) * 128:(kh * KH + mh + 1) * 128],
                        cs[:, kh, :], start=(kh == 0), stop=(kh == KH - 1))
                nc.vector.tensor_copy(cw[:, mh, :], pcw)

            def fold_vec(vT, blk, tag):
                # (w1[:, blk] @ v) -> [BL, A] bf16 (lhsT for b-onehot matmul)
                pf = attps.tile([BL, A], f32, tag="ps")
                for kh in range(KH):
                    nc.tensor.matmul(pf, vT[:, 4 * kh:4 * kh + 4],
                                     w1b[:, (blk * KH + kh) * A:(blk * KH + kh + 1) * A],
                                     start=(kh == 0), stop=(kh == KH - 1))
                sb = att.tile([BL, A], bf16, tag=tag)
                nc.vector.tensor_copy(sb[:], pf)
                return sb

            qhf = fold_vec(qh, 2, "qhf")

            def dots(vT, dst):
                # dst[0, (b,n)] = sum_h cw[h,(b,n)] * v[h,b]
                pd = attps.tile([1, BL * N], f32, tag="ps")
                for b in range(BL):
                    for kh in range(KH):
                        nc.tensor.matmul(
                            pd[:, b * N:(b + 1) * N],
                            vT[:, 4 * kh + b:4 * kh + b + 1],
                            cw[:, kh, b * N:(b + 1) * N],
                            start=(kh == 0), stop=(kh == KH - 1))
                nc.vector.tensor_copy(dst[:], pd)

            dq_sb = att.tile([1, BL * N], bf16, tag="dq_sb")
            dm_sb = att.tile([1, BL * N], bf16, tag="dm_sb")
            dots(qh, dq_sb)

            def bcast_kh(vT):
                # [128, 8] -> virtual [128, KH, BL, N] (stride-0 over n)
                return _ap(vT[:], 0, [[4, KH], [1, BL], [0, N]])

            csv = cs[:].rearrange("p kh (b n) -> p kh b n", b=BL)
            csq = att.tile([128, KH, BL, N], bf16, tag="csq")
            nc.vector.tensor_tensor(csq, csv, bcast_kh(qh), ALU.mult)
            dq_ = att.tile([128, KH, BL, N], bf16, tag="dq_")
            nc.vector.tensor_tensor(dq_, csv, bcast_kh(qh), ALU.subtract)
            acsq = att.tile([128, KH, BL, N], bf16, tag="acsq")
            nc.vector.scalar_tensor_tensor(acsq, dq_, -1.0, dq_, ALU.mult, ALU.max)

            m_cur = qh
            # ---- hops ----
            for hop in range(MEM_COUNT):
                mf = fold_vec(m_cur, 1, "mf")
                dots(m_cur, dm_sb)
                csm = att.tile([128, KH, BL, N], bf16, tag="csm")
                nc.vector.tensor_tensor(csm, csv, bcast_kh(m_cur), ALU.mult)
                dm_ = att.tile([128, KH, BL, N], bf16, tag="dm_")
                nc.vector.tensor_tensor(dm_, csv, bcast_kh(m_cur), ALU.subtract)
                acsm = att.tile([128, KH, BL, N], bf16, tag="acsm")
                nc.vector.scalar_tensor_tensor(acsm, dm_, -1.0, dm_, ALU.mult, ALU.max)

                # h1 = tanh(feats @ w1.T + b1): accumulate [A, 512]
                ph1 = attps.tile([A, BL * N], f32, tag="ps")
                first = True
                for blk, src_t in ((0, cs), (3, csq), (4, csm), (5, acsq), (6, acsm)):
                    for kh in range(KH):
                        nc.tensor.matmul(
                            ph1, w1b[:, (blk * KH + kh) * A:(blk * KH + kh + 1) * A],
                            src_t[:, kh].rearrange("p b n -> p (b n)") if src_t is not cs
                            else cs[:, kh, :],
                            start=first, stop=False, skip_group_check=True)
                        first = False
                nc.tensor.matmul(ph1, w1dq[:], dq_sb[:], start=False, stop=False,
                                 skip_group_check=True)
                nc.tensor.matmul(ph1, w1dm[:], dm_sb[:], start=False, stop=False,
                                 skip_group_check=True)
                nc.tensor.matmul(ph1, qhf[:], bonehot[:], start=False, stop=False,
                                 skip_group_check=True)
                nc.tensor.matmul(ph1, mf[:], bonehot[:], start=False, stop=True,
                                 skip_group_check=True)
                h1s = att.tile([A, BL * N], bf16, tag="h1s")
                nc.scalar.activation(h1s, ph1, AF.Tanh, bias=b1[:])
                if io["_dbg"] and hop == 0:
                    nc.sync.dma_start(out=io["dbg_h1"], in_=h1s[:])

                # scores -> masked softmax over n
                psc = attps.tile([1, BL * N], f32, tag="ps")
                nc.tensor.matmul(psc, w2[:], h1s[:], start=True, stop=True)
                sc = att.tile([1, BL * N], f32, tag="sc")
                nc.vector.scalar_tensor_tensor(sc, psc, b2[:, 0:1], maskadd[:],
                                               ALU.add, ALU.add)
                pe = att.tile([1, BL * N], f32, tag="pe")
                nc.scalar.activation(pe, sc, AF.Exp)
                ssum = att.tile([1, BL], f32, tag="ssum")
                nc.vector.tensor_reduce(
                    ssum, pe[:].rearrange("p (b n) -> p b n", b=BL),
                    axis=mybir.AxisListType.X, op=ALU.add)
                srec = att.tile([1, BL], f32, tag="srec")
                nc.vector.reciprocal(srec, ssum)
                p_ = att.tile([1, BL * N], f32, tag="p_")
                nc.vector.tensor_tensor(
                    p_.rearrange("p (b n) -> p b n", b=BL),
                    pe[:].rearrange("p (b n) -> p b n", b=BL),
                    _ap(srec[:], 0, [[1, BL], [0, N]]), ALU.mult)
                dstp = bass.AP(tensor=io["att_out"].tensor,
                               offset=io["att_out"].offset + hop * BL * N,
                               ap=[[BL * N, 1], [1, BL * N]])
                nc.sync.dma_start(out=dstp, in_=p_[:])

                # e = sum_n p * c_sel  -> [128, 8] bf16
                ppb = attps.tile([128, BL * N], f32, tag="ps")
                nc.tensor.matmul(ppb, ones_k1[:], p_[:], start=True, stop=True)
                pbs = att.tile([128, BL * N], bf16, tag="pbs")
                nc.vector.tensor_copy(pbs[:], ppb)
                tmpe = att.tile([128, KH, BL, N], f32, tag="tmpe")
                nc.vector.tensor_tensor(
                    tmpe, csv, _ap(pbs[:], 0, [[0, KH], [N, BL], [1, N]]), ALU.mult)
                ef = att.tile([128, KH, BL], f32, tag="ef")
                nc.vector.tensor_reduce(ef, tmpe[:], axis=mybir.AxisListType.X,
                                        op=ALU.add)
                eT = att.tile([128, 8], bf16, tag="eT")
                nc.vector.tensor_copy(eT[:], ef[:].rearrange("p kh b -> p (kh b)"))
                if io["_dbg"] and hop == 0:
                    de0 = att.tile([128, 8], f32, tag="de0")
                    nc.vector.tensor_copy(de0[:], eT[:])
                    nc.sync.dma_start(out=io["dbg_e0"], in_=de0[:])

                # mem GRU cell: m_new = GRUCell(e, m)
                pm = attps.tile([128, 32], f32, tag="ps")
                nc.tensor.matmul(pm[:, 0:32], ident[:], memb[:],
                                 start=True, stop=False, skip_group_check=True)
                for mt in range(NMT):
                    col = 4 * mt if mt < 4 else 16 + 4 * (mt - 4)
                    for kh in range(KH):
                        nc.tensor.matmul(
                            pm[:, col:col + 4],
                            memw_ih[:, (kh * NMT + mt) * 128:(kh * NMT + mt + 1) * 128],
                            eT[:, 4 * kh:4 * kh + 4],
                            start=False, stop=False, skip_group_check=True)
                for mt in range(NMT):
                    col = 4 * mt if mt < 4 else 24 + 4 * (mt - 4)
                    for kh in range(KH):
                        nc.tensor.matmul(
                            pm[:, col:col + 4],
                            memw_hh[:, (kh * NMT + mt) * 128:(kh * NMT + mt + 1) * 128],
                            m_cur[:, 4 * kh:4 * kh + 4],
                            start=False, stop=(mt == NMT - 1 and kh == KH - 1),
                            skip_group_check=True)
                m_new = att.tile([128, 8], bf16, tag=f"m{hop}")
                _emit_gru_cell_gates_mem(nc, work, pm, m_cur, m_new, f"m{hop}")
                if io["_dbg"] and hop == 0:
                    dm1 = att.tile([128, 8], f32, tag="dm1")
                    nc.vector.tensor_copy(dm1[:], m_new[:])
                    nc.sync.dma_start(out=io["dbg_m1"], in_=dm1[:])
                m_cur = m_new

            # ---- answer module ----
            msq = m_cur
            for it in range(ANS_COUNT + 1):
                py = attps.tile([128, OT * BL], f32, tag="ps")
                nc.tensor.matmul(py[:, 0:OT * BL], ident[:], outb_bc[:],
                                 start=True, stop=False, skip_group_check=True)
                for mo in range(OT):
                    for kh in range(KH):
                        nc.tensor.matmul(
                            py[:, mo * BL:(mo + 1) * BL],
                            outw[:, (kh * OT + mo) * 128:(kh * OT + mo + 1) * 128],
                            msq[:, 4 * kh:4 * kh + 4],
                            start=False, stop=(kh == KH - 1), skip_group_check=True)
                yexp = att.tile([128, OT, BL], f32, tag="yexp")
                nc.scalar.activation(yexp, py[:].rearrange("p (mo b) -> p mo b", b=BL),
                                     AF.Exp)
                rsum = att.tile([128, BL], f32, tag="rsum")
                nc.vector.tensor_reduce(
                    rsum, _ap(yexp[:], 0, [[1, BL], [BL, OT]]),
                    axis=mybir.AxisListType.X, op=ALU.add)
                pcs = attps.tile([1, BL], f32, tag="ps")
                nc.tensor.matmul(pcs, ones_col[:], rsum[:], start=True, stop=True)
                yrec = att.tile([1, BL], f32, tag="yrec")
                nc.vector.reciprocal(yrec, pcs)
                prb = attps.tile([128, BL], f32, tag="ps")
                nc.tensor.matmul(prb, ones_k1[:], yrec[:], start=True, stop=True)
                if it == ANS_COUNT:
                    yf = att.tile([128, OT, BL], f32, tag="yf")
                    nc.vector.tensor_tensor(yf, yexp[:],
                                            _ap(prb[:], 0, [[0, OT], [1, BL]]),
                                            ALU.mult)
                    for b in range(BL):
                        for mo in range(OT):
                            rows = 128 if mo < OT - 1 else OT_LAST
                            dsty = bass.AP(
                                tensor=io["y_out"].tensor,
                                offset=io["y_out"].offset + b * O + mo * 128,
                                ap=[[1, rows], [1, 1]])
                            nc.sync.dma_start(
                                out=dsty,
                                in_=_app(yf[:], 0, rows, mo * BL + b, [[1, 1]]))
                else:
                    yn = att.tile([128, OT, BL], bf16, tag="yn")
                    nc.vector.tensor_tensor(yn, yexp[:],
                                            _ap(prb[:], 0, [[0, OT], [1, BL]]),
                                            ALU.mult)
                    pa = attps.tile([128, 32], f32, tag="ps")
                    nc.tensor.matmul(pa[:, 0:32], ident[:], ansb[:],
                                     start=True, stop=False, skip_group_check=True)
                    for mt in range(NMT):
                        col = 4 * mt if mt < 4 else 16 + 4 * (mt - 4)
                        for kt in range(16):
                            nc.tensor.matmul(
                                pa[:, col:col + 4],
                                answ_ih[:, (kt * NMT + mt) * 128:(kt * NMT + mt + 1) * 128],
                                yn[:, kt, :],
                                start=False, stop=False, skip_group_check=True)
                        for kh in range(KH):
                            nc.tensor.matmul(
                                pa[:, col:col + 4],
                                answ_ih[:, ((16 + kh) * NMT + mt) * 128:((16 + kh) * NMT + mt + 1) * 128],
                                qh[:, 4 * kh:4 * kh + 4],
                                start=False, stop=False, skip_group_check=True)
                    for mt in range(NMT):
                        col = 4 * mt if mt < 4 else 24 + 4 * (mt - 4)
                        for kh in range(KH):
                            nc.tensor.matmul(
                                pa[:, col:col + 4],
                                answ_hh[:, (kh * NMT + mt) * 128:(kh * NMT + mt + 1) * 128],
                                msq[:, 4 * kh:4 * kh + 4],
                                start=False, stop=(mt == NMT - 1 and kh == KH - 1),
                                skip_group_check=True)
                    msq_new = att.tile([128, 8], bf16, tag=f"msq{it}")
                    _emit_gru_cell_gates_mem(nc, work, pa, msq, msq_new, f"a{it}")
                    msq = msq_new


def _emit_gru_cell_gates_mem(nc, work, ps, hprev, hout, tag):
    """Gate math for psum layout [0:16 rz | 16:24 gin | 24:32 ghn]."""
    rz = work.tile([128, 16], f32, tag=f"rz{tag}")
    nc.scalar.activation(rz, ps[:, 0:16], AF.Sigmoid)
    zc = work.tile([128, 8], f32, tag=f"zc{tag}")
    nc.scalar.activation(zc, ps[:, 8:16], AF.Sigmoid, scale=-1.0)
    t1 = work.tile([128, 8], bf16, tag=f"t1{tag}")
    nc.vector.tensor_tensor(t1, ps[:, 24:32], rz[:, 0:8], ALU.mult)
    npre = work.tile([128, 8], bf16, tag=f"np{tag}")
    nc.vector.tensor_tensor(npre, ps[:, 16:24], t1, ALU.add)
    n_ = work.tile([128, 8], f32, tag=f"n{tag}")
    nc.scalar.activation(n_, npre, AF.Tanh)
    zh = work.tile([128, 8], f32, tag=f"zh{tag}")
    nc.vector.tensor_tensor(zh, hprev[:], rz[:, 8:16], ALU.mult)
    t2 = work.tile([128, 8], f32, tag=f"t2{tag}")
    nc.vector.tensor_tensor(t2, n_, zc, ALU.mult)
    nc.vector.tensor_tensor(hout[:], t2, zh, ALU.add)


# ---------------------------------------------------------------------------
# host-side prep
# ---------------------------------------------------------------------------

def _gate_tiles_ih(w):  # w [768, K] -> [128(K... per-tile k), NMT*128]
    K = w.shape[1]
    assert K == 128
    out = np.zeros((128, NMT * 128), np.float32)
    for mt in range(NMT):
        out[:, mt * 128:(mt + 1) * 128] = w[mt * 128:(mt + 1) * 128, :].T
    return out


def _gate_tiles_k(w, nkt=None):  # w [768, K], K = nkt*128 (zero-pad) -> [128, nkt*NMT*128]
    K = w.shape[1]
    nkt = nkt or (K + 127) // 128
    wp = np.zeros((G3, nkt * 128), np.float32)
    wp[:, :K] = w
    out = np.zeros((128, nkt * NMT * 128), np.float32)
    for kt in range(nkt):
        for mt in range(NMT):
            out[:, (kt * NMT + mt) * 128:(kt * NMT + mt + 1) * 128] = \
                wp[mt * 128:(mt + 1) * 128, kt * 128:(kt + 1) * 128].T
    return out


def _gru_consts(pfx, b_ih, b_hh):
    gib = np.zeros((128, NMT), np.float32)
    for mt in range(NMT):
        gib[:, mt] = b_ih[mt * 128:(mt + 1) * 128]
        if mt < 4:
            gib[:, mt] += b_hh[mt * 128:(mt + 1) * 128]
    bhhn = np.zeros((128, 8), np.float32)
    for i in range(2):
        for b in range(BL):
            bhhn[:, i * 4 + b] = b_hh[(4 + i) * 128:(5 + i) * 128]
    return {f"gib_{pfx}": gib, f"bhhn_{pfx}": _bf(bhhn)}


def _cell_bias(b_ih, b_hh):
    out = np.zeros((128, 32), np.float32)
    for mt in range(4):
        for b in range(BL):
            out[:, mt * 4 + b] = (b_ih + b_hh)[mt * 128:(mt + 1) * 128]
    for i in range(2):
        for b in range(BL):
            out[:, 16 + i * 4 + b] = b_ih[(4 + i) * 128:(5 + i) * 128]
            out[:, 24 + i * 4 + b] = b_hh[(4 + i) * 128:(5 + i) * 128]
    return out


def _bf(x):
    import ml_dtypes
    return np.asarray(x, np.float32).astype(ml_dtypes.bfloat16)


def prep_inputs(inputs, S=S_FULL, SQ=SQ_FULL):
    """Returns (in_maps list per core)."""
    ii = {k: np.asarray(v) for k, v in inputs.items()}
    shared = {}
    shared["wih_c"] = _bf(_gate_tiles_ih(ii["in_w_ih"]))
    shared["whh_c"] = _bf(_gate_tiles_k(ii["in_w_hh"]))
    shared["wih_q"] = _bf(_gate_tiles_ih(ii["qe_w_ih"]))
    shared["whh_q"] = _bf(_gate_tiles_k(ii["qe_w_hh"]))
    shared.update(_gru_consts("c", ii["in_b_ih"], ii["in_b_hh"]))
    shared.update(_gru_consts("q", ii["qe_b_ih"], ii["qe_b_hh"]))
    shared["ident"] = _bf(np.eye(128, dtype=np.float32))
    shared["ones_k1"] = np.ones((1, 128), np.float32)
    shared["ones_col"] = np.ones((128, 1), np.float32)
    # att_weight [H, H] -> lhsT tiles [k, (kh*KH+mh)*128+m]
    aw = np.zeros((128, KH * KH * 128), np.float32)
    for kh in range(KH):
        for mh in range(KH):
            aw[:, (kh * KH + mh) * 128:(kh * KH + mh + 1) * 128] = \
                ii["att_weight"][kh * 128:(kh + 1) * 128, mh * 128:(mh + 1) * 128]
    shared["attw"] = _bf(aw)
    w1 = ii["att_w1"]  # [A, 7H+2]
    w1b = np.zeros((128, 7 * KH * A), np.float32)
    for blk in range(7):
        for kh in range(KH):
            w1b[:, (blk * KH + kh) * A:(blk * KH + kh + 1) * A] = \
                w1[:, blk * H + kh * 128:blk * H + (kh + 1) * 128].T
    shared["w1b"] = _bf(w1b)
    shared["w1dq"] = _bf(w1[:, 7 * H:7 * H + 1].T)
    shared["w1dm"] = _bf(w1[:, 7 * H + 1:7 * H + 2].T)
    shared["b1"] = np.asarray(ii["att_b1"], np.float32).reshape(A, 1)
    shared["w2"] = _bf(ii["att_w2"].reshape(1, A).T)
    shared["b2"] = np.asarray(ii["att_b2"], np.float32).reshape(1, 1)
    shared["memw_ih"] = _bf(_gate_tiles_k(ii["mem_w_ih"]))
    shared["memw_hh"] = _bf(_gate_tiles_k(ii["mem_w_hh"]))
    shared["memb"] = _bf(_cell_bias(ii["mem_b_ih"], ii["mem_b_hh"]))
    # ans_w_ih [768, O+H]: y part zero-padded to 16*128, then qh part
    awi = ii["ans_w_ih"]
    awi_p = np.zeros((G3, 18 * 128), np.float32)
    awi_p[:, :O] = awi[:, :O]
    awi_p[:, 16 * 128:16 * 128 + H] = awi[:, O:]
    shared["answ_ih"] = _bf(_gate_tiles_k(awi_p))
    shared["answ_hh"] = _bf(_gate_tiles_k(ii["ans_w_hh"]))
    shared["ansb"] = _bf(_cell_bias(ii["ans_b_ih"], ii["ans_b_hh"]))
    ow = np.zeros((128, KH * OT * 128), np.float32)
    owp = np.zeros((OT * 128, H), np.float32)
    owp[:O] = ii["out_w"]
    for kh in range(KH):
        for mo in range(OT):
            ow[:, (kh * OT + mo) * 128:(kh * OT + mo + 1) * 128] = \
                owp[mo * 128:(mo + 1) * 128, kh * 128:(kh + 1) * 128].T
    shared["outw"] = _bf(ow)
    ob = np.zeros((128, OT * BL), np.float32)
    obp = np.full(OT * 128, NEG, np.float32)
    obp[:O] = ii["out_b"]
    for mo in range(OT):
        for b in range(BL):
            ob[:, mo * BL + b] = obp[mo * 128:(mo + 1) * 128]
    shared["outb_bc"] = _bf(ob)

    c = np.asarray(ii["c"], np.float32)[:, :S]
    q = np.asarray(ii["q"], np.float32)[:, :SQ]
    cidx = np.asarray(ii["c_index"]).astype(np.int64)
    lc = np.asarray(ii["len_c"]).astype(np.int64)
    i_state = np.asarray(ii["i_state"], np.float32)
    q_state = np.asarray(ii["q_state"], np.float32)

    in_maps = []
    for core in range(NCORES):
        b0 = core * BL
        m = dict(shared)
        # cT[d, t*BL+b] = c[b0+b, t, d]
        m["cT"] = _bf(c[b0:b0 + BL].transpose(2, 1, 0).reshape(DIN, S * BL))
        m["qT"] = _bf(q[b0:b0 + BL].transpose(2, 1, 0).reshape(DIN, SQ * BL))
        # h0[p, kh*BL+b] = state[0, b0+b, kh*128+p]
        m["h0c"] = _bf(i_state[0, b0:b0 + BL].reshape(BL, KH, 128)
                       .transpose(2, 1, 0).reshape(128, KH * BL))
        m["h0q"] = _bf(q_state[0, b0:b0 + BL].reshape(BL, KH, 128)
                       .transpose(2, 1, 0).reshape(128, KH * BL))
        # gather rows: row index b*S + c_index[b0+b, n], order k=b*N+n
        rows = (np.arange(BL)[:, None] * S + cidx[b0:b0 + BL]).reshape(-1)
        gw = np.zeros((16, 32), np.int16)
        for k in range(BL * N):
            gw[k % 16, k // 16] = rows[k]
        m["gidx"] = np.tile(gw, (8, 1))
        mask = np.zeros((1, BL * N), np.float32)
        for b in range(BL):
            mask[0, b * N:(b + 1) * N] = np.where(np.arange(N) < lc[b0 + b], 0.0, NEG)
        m["maskadd"] = mask
        bo = np.zeros((BL, BL * N), np.float32)
        for b in range(BL):
            bo[b, b * N:(b + 1) * N] = 1.0
        m["bonehot"] = _bf(bo)
        in_maps.append(m)
    return in_maps


_CACHE = {}


def kernel(**inputs):
    key = "full"
    if key not in _CACHE:
        _CACHE[key] = build_nc()
    nc, _ = _CACHE[key]
    in_maps = prep_inputs(inputs)
    res = None
    for attempt in range(3):
        try:
            res = run_bass_kernel_spmd(nc, in_maps, list(range(NCORES)))
            break
        except Exception:
            if attempt == 2:
                raise
            import time as _time
            _time.sleep(25)
    y = np.concatenate([res.results[i]["y_out"] for i in range(NCORES)], axis=0)
    att = np.concatenate([res.results[i]["att_out"] for i in range(NCORES)], axis=1)
    return np.asarray(y, np.float32), np.asarray(att, np.float32)[..., None]
